# revision 13
# baseline (speedup 1.0000x reference)
"""ChebGCN (K=2, 3 layers) Trainium2 kernel — 8-core SPMD.

Sharding: nodes are split across 8 cores (12500/core). Within a core,
local nodes are PERMUTED so they are grouped by degree bucket (host-side
renumbering; inputs/outputs are permuted on the host for free). Each
node's incoming edges occupy a fixed per-bucket budget of M slots
(M in {12,16,20,24,tail}, zero-padded), so the whole propagate becomes:
indirect-DMA gather of 128 source rows per (tile, occurrence) slot column,
one broadcast tensor_tensor to apply edge weights, and ONE tensor_reduce
over the innermost occurrence axis per tile group — no per-chunk matmuls.
Per layer the reduced segment sums are stored row-major by transposed-AP
DMA, re-transposed whole-table by one DMA-transpose for the dense term,
and the dense 64-wide weight matmuls + bias/relu run 4 node-tiles (512
cols) per op in transposed layout with running features resident in SBUF.
An 8-core AllGather rebuilds the global row table per layer.

Host->device traffic is one int32 tensor per core packing, bit-cast per
region: per-slot source indices (int32) and weights (bf16), the permuted
bf16 x slice, and weight/bias constants. Output leaves as bf16 and is
un-permuted on the host.
"""

import sys

for _p in ("/opt/trn_rl_repo",):
    if _p not in sys.path:
        sys.path.insert(0, _p)

import math
import time
from contextlib import ExitStack

import ml_dtypes
import numpy as np

import concourse.bacc as bacc
import concourse.bass as bass
import concourse.mybir as mybir
import concourse.tile as tile
from concourse.bass_utils import run_bass_kernel_spmd

F32 = mybir.dt.float32
I32 = mybir.dt.int32
BF16 = mybir.dt.bfloat16
NP_BF16 = ml_dtypes.bfloat16

M_CORES = 8
TG = 4        # node tiles per dense matmul batch (512 cols)
XW = 32       # int32 cols per x tile in the blob (64 bf16 feats)
TMAX = 96     # max slot-columns (Tt*M) per gather/reduce group
CW = 292      # bf16 const-region cols (even)
W10_C, W11_C, WX0_C, WX1_C = 0, 64, 128, 192
W20_C, W21_C, B1_C, BX_C, B2_C = 256, 272, 288, 289, 290
LAST_TIMES = []  # wall times of repeat runs (filled by run(timeit=N))


# ---------------------------------------------------------------- host prep
def host_prep(adj, n_nodes, npc):
    """Degree-bucket nodes per core, build slot tables.

    Returns dict with: groups [(t0, Tt, M, col0)], tile_m, n_tiles2, tw,
    per_core [{offs [128,TW] i32, wgt [128,TW] f32}], new_of_old [8][npc].
    """
    row = adj[0].astype(np.int64)
    col = adj[1].astype(np.int64)
    deg = np.bincount(row, minlength=n_nodes).astype(np.int64)
    dis = np.where(deg > 0, 1.0 / np.sqrt(np.maximum(deg, 1)), 0.0).astype(
        np.float32
    )
    w_all = (-(dis[row] * dis[col])).astype(np.float32)

    maxdeg = int(deg.max())
    ms = [m for m in (6, 8, 10, 12, 14, 16, 18, 20, 22, 24, 26, 28, 32)
          if m < maxdeg]
    ms.append(max(maxdeg, (ms[-1] + 4) if ms else 12))
    nb = len(ms)
    ms_arr = np.array(ms)

    def bucket_of(d):
        return np.searchsorted(ms_arr, np.maximum(d, 1))

    b_of = [bucket_of(deg[c * npc:(c + 1) * npc]) for c in range(M_CORES)]
    counts = np.zeros((M_CORES, nb), dtype=np.int64)
    for c in range(M_CORES):
        counts[c] = np.bincount(b_of[c], minlength=nb)
    nb_tiles = np.array(
        [int(math.ceil(counts[:, b].max() / 128.0)) for b in range(nb)]
    )
    n_tiles2 = int(nb_tiles.sum())
    npcp2 = 128 * n_tiles2

    tile_m = []
    for b in range(nb):
        tile_m += [ms[b]] * int(nb_tiles[b])
    col0 = np.concatenate([[0], np.cumsum(tile_m)[:-1]]).astype(np.int64)
    tw = int(np.sum(tile_m))

    groups = []
    t = 0
    while t < n_tiles2:
        m = tile_m[t]
        tt = max(1, TMAX // m)
        tt = min(tt, n_tiles2 - t)
        while tile_m[t + tt - 1] != m:
            tt -= 1
        groups.append((t, tt, m, int(col0[t])))
        t += tt

    base_b = 128 * np.concatenate([[0], np.cumsum(nb_tiles)[:-1]])
    new_of_old = []
    for c in range(M_CORES):
        order = np.argsort(b_of[c], kind="stable")
        noo = np.zeros(npc, dtype=np.int64)
        pos_in_b = np.zeros(nb, dtype=np.int64)
        sorted_b = b_of[c][order]
        # positions within each bucket follow sorted order
        starts = np.searchsorted(sorted_b, np.arange(nb))
        noo[order] = base_b[sorted_b] + (np.arange(npc) - starts[sorted_b])
        new_of_old.append(noo)

    # global padded source index per edge
    sc = col // npc
    sl = col % npc
    noo_all = np.stack(new_of_old)  # [8, npc]
    colp = sc * npcp2 + noo_all[sc, sl]

    per_core = []
    core_of = row // npc
    for c in range(M_CORES):
        sel = np.nonzero(core_of == c)[0]
        d_loc = row[sel] - c * npc
        q = new_of_old[c][d_loc]
        order = np.argsort(q, kind="stable")
        sel = sel[order]
        qs = q[order]
        # occurrence j within each destination's slot budget
        seg_start = np.searchsorted(qs, qs)
        j = np.arange(len(qs)) - seg_start
        t_of = qs // 128
        p_of = qs % 128
        cols = col0[t_of] + j
        offs = np.zeros((128, tw), dtype=np.int32)
        wgt = np.zeros((128, tw), dtype=np.float32)
        offs[p_of, cols] = colp[sel]
        wgt[p_of, cols] = w_all[sel]
        per_core.append(dict(offs=offs, wgt=wgt))

    return dict(groups=groups, tile_m=tile_m, n_tiles2=n_tiles2, tw=tw,
                per_core=per_core, new_of_old=new_of_old, npcp2=npcp2)


def blob_geom(tw, n_tiles2):
    tw2 = tw + (tw & 1)
    xb = tw                      # x region start (i32 cols)
    cb = xb + n_tiles2 * XW      # const region
    wb = cb + CW // 2            # weight region
    w32 = wb + tw2 // 2
    return xb, cb, wb, w32


# ------------------------------------------------------------- bass program
def build_program(hp, fin, fhid, fout):
    groups = hp["groups"]
    n_tiles2 = hp["n_tiles2"]
    tw = hp["tw"]
    npcp2 = hp["npcp2"]
    np_all = npcp2 * M_CORES
    tw2 = tw + (tw & 1)
    xb, cbase, wbase, w32 = blob_geom(tw, n_tiles2)

    nc = bacc.Bacc(
        "TRN2",
        target_bir_lowering=False,
        debug=False,
        enable_asserts=False,
        num_devices=M_CORES,
    )

    blob_d = nc.dram_tensor("blob", [128, w32], I32, kind="ExternalInput")
    out_d = nc.dram_tensor("out", [fout, npcp2], BF16, kind="ExternalOutput")

    xrows = nc.dram_tensor("xrows", [npcp2, fin], BF16)
    rows1 = nc.dram_tensor("rows1", [npcp2, fhid], BF16)
    rows2 = nc.dram_tensor("rows2", [npcp2, fhid], BF16)
    tx1_d = nc.dram_tensor("tx1", [npcp2, fhid], BF16)
    tx2_d = nc.dram_tensor("tx2", [npcp2, fhid], BF16)
    tx3_d = nc.dram_tensor("tx3", [npcp2, fhid], BF16)
    tab1 = nc.dram_tensor("tab1", [np_all, fin], BF16, addr_space="Shared")
    tab2 = nc.dram_tensor("tab2", [np_all, fhid], BF16, addr_space="Shared")
    tab3 = nc.dram_tensor("tab3", [np_all, fhid], BF16, addr_space="Shared")

    rg = [list(range(M_CORES))]

    with ExitStack() as ctx:
        tc = ctx.enter_context(tile.TileContext(nc))
        const = ctx.enter_context(tc.tile_pool(name="const", bufs=1))
        rp = ctx.enter_context(tc.tile_pool(name="rp", bufs=2))
        rwp = ctx.enter_context(tc.tile_pool(name="rwp", bufs=2))
        txop = ctx.enter_context(tc.tile_pool(name="txop", bufs=2))
        otp = ctx.enter_context(tc.tile_pool(name="otp", bufs=2))
        psB = ctx.enter_context(tc.tile_pool(name="psB", bufs=2, space="PSUM"))

        # const region: one DMA, then slice views
        cb = const.tile([128, CW], BF16, tag="cb")
        nc.sync.dma_start(
            out=cb[:],
            in_=blob_d[:, cbase:cbase + CW // 2].bitcast(BF16),
        )
        w10_t = cb[0:fin, W10_C:W10_C + fhid]
        w11_t = cb[0:fin, W11_C:W11_C + fhid]
        wx0_t = cb[0:fhid, WX0_C:WX0_C + fhid]
        wx1_t = cb[0:fhid, WX1_C:WX1_C + fhid]
        w20_t = cb[0:fhid, W20_C:W20_C + fout]
        w21_t = cb[0:fhid, W21_C:W21_C + fout]
        b1_t = const.tile([fhid, 1], F32, tag="b1")
        nc.vector.tensor_copy(out=b1_t[:], in_=cb[0:fhid, B1_C:B1_C + 1])
        bx_t = const.tile([fhid, 1], F32, tag="bx")
        nc.vector.tensor_copy(out=bx_t[:], in_=cb[0:fhid, BX_C:BX_C + 1])
        b2_t = const.tile([fout, 1], F32, tag="b2")
        nc.vector.tensor_copy(out=b2_t[:], in_=cb[0:fout, B2_C:B2_C + 1])

        # slot metadata: two bulk DMAs, no unpacking needed
        offs_all = const.tile([128, tw], I32, tag="offs")
        nc.sync.dma_start(out=offs_all[:], in_=blob_d[:, 0:tw])
        w_sb = const.tile([128, tw2], BF16, tag="wal")
        nc.sync.dma_start(
            out=w_sb[:],
            in_=blob_d[:, wbase:wbase + tw2 // 2].bitcast(BF16),
        )

        # x prologue: bulk load, row-major store, AllGather, transpose
        xa = const.tile([128, n_tiles2 * XW], I32, tag="xa")
        nc.sync.dma_start(out=xa[:],
                          in_=blob_d[:, xb:xb + n_tiles2 * XW])
        xa16 = xa[:].bitcast(BF16).rearrange("p (t f) -> p t f", f=2 * XW)
        nc.sync.dma_start(
            out=xrows[:, :].rearrange("(t p) f -> p t f", p=128),
            in_=xa16,
        )
        nc.gpsimd.collective_compute(
            "AllGather",
            mybir.AluOpType.bypass,
            replica_groups=rg,
            ins=[xrows[:, :]],
            outs=[tab1[:, :]],
        )
        xT_sb = const.tile([fin, npcp2], BF16, tag="xT")
        nc.sync.dma_start_transpose(xT_sb[:], xrows[:, :])

        txT_sb = const.tile([fhid, npcp2], BF16, tag="txT")
        hT1_sb = const.tile([fhid, npcp2], BF16, tag="hT1")
        hT2_sb = const.tile([fhid, npcp2], BF16, tag="hT2")

        layers = [
            dict(table=tab1, rhs_sb=xT_sb, W0=w10_t, W1=w11_t, b=b1_t,
                 fo=fhid, hT_next=hT1_sb, rows=rows1, tab_next=tab2,
                 tx_d=tx1_d),
            dict(table=tab2, rhs_sb=hT1_sb, W0=wx0_t, W1=wx1_t, b=bx_t,
                 fo=fhid, hT_next=hT2_sb, rows=rows2, tab_next=tab3,
                 tx_d=tx2_d),
            dict(table=tab3, rhs_sb=hT2_sb, W0=w20_t, W1=w21_t, b=b2_t,
                 fo=fout, hT_next=None, rows=None, tab_next=None,
                 tx_d=tx3_d),
        ]

        for li, L in enumerate(layers):
            fo = L["fo"]
            # propagate: gather / weight / reduce per tile group
            for (t0, tt, m, c0) in groups:
                r = rp.tile([128, tt, m, fin], BF16, tag="r")
                for t in range(tt):
                    for j in range(m):
                        nc.gpsimd.indirect_dma_start(
                            out=r[:, t, j, :],
                            out_offset=None,
                            in_=L["table"][:, :],
                            in_offset=bass.IndirectOffsetOnAxis(
                                ap=offs_all[:, c0 + t * m + j:
                                            c0 + t * m + j + 1],
                                axis=0,
                            ),
                        )
                rw = rwp.tile([128, tt, m, fin], BF16, tag="rw")
                nc.vector.tensor_tensor(
                    out=rw[:],
                    in0=r[:],
                    in1=w_sb[:, c0:c0 + tt * m].rearrange(
                        "p (t m) -> p t m", m=m
                    ).unsqueeze(3).broadcast_to([128, tt, m, fin]),
                    op=mybir.AluOpType.mult,
                )
                txo = txop.tile([128, tt, fin], BF16, tag="txo")
                with nc.allow_low_precision(
                    reason="segment sums of <=40 bf16 terms; tolerance 2e-2"
                ):
                    nc.vector.tensor_reduce(
                        out=txo[:],
                        in_=rw[:].rearrange("p t m f -> p t f m"),
                        axis=mybir.AxisListType.X,
                        op=mybir.AluOpType.add,
                    )
                nc.sync.dma_start(
                    out=L["tx_d"][t0 * 128:(t0 + tt) * 128, :].rearrange(
                        "(t p) f -> p t f", p=128
                    ),
                    in_=txo[:],
                )
            # whole-layer transpose of segment sums for the dense term
            nc.sync.dma_start_transpose(txT_sb[:], L["tx_d"][:, :])
            # dense + bias/relu, TG node tiles per op
            for q0 in range(0, n_tiles2, TG):
                qt = min(TG, n_tiles2 - q0)
                c0 = q0 * 128
                gw = qt * 128
                pb = psB.tile([fo, TG * 128], F32, tag="pb")
                nc.tensor.matmul(
                    pb[:, :gw], lhsT=L["W0"],
                    rhs=L["rhs_sb"][:, c0:c0 + gw],
                    start=True, stop=False,
                )
                nc.tensor.matmul(
                    pb[:, :gw], lhsT=L["W1"],
                    rhs=txT_sb[:, c0:c0 + gw],
                    start=False, stop=True,
                )
                if L["hT_next"] is not None:
                    osl = L["hT_next"][:, c0:c0 + gw]
                    nc.scalar.activation(
                        osl, pb[:, :gw],
                        mybir.ActivationFunctionType.Relu,
                        bias=L["b"][:],
                    )
                    nc.sync.dma_start(
                        out=L["rows"][c0:c0 + gw, :].rearrange(
                            "n f -> f n"
                        ),
                        in_=osl,
                    )
                else:
                    ot = otp.tile([fout, TG * 128], BF16, tag="ot")
                    nc.scalar.activation(
                        ot[:, :gw], pb[:, :gw],
                        mybir.ActivationFunctionType.Identity,
                        bias=L["b"][:],
                    )
                    nc.sync.dma_start(
                        out=out_d[:, c0:c0 + gw], in_=ot[:, :gw]
                    )
            if L["tab_next"] is not None:
                nc.gpsimd.collective_compute(
                    "AllGather",
                    mybir.AluOpType.bypass,
                    replica_groups=rg,
                    ins=[L["rows"][:, :]],
                    outs=[L["tab_next"][:, :]],
                )

    nc.compile()
    return nc


# ------------------------------------------------------------------ runner
def make_in_maps(inputs, n_nodes, npc, hp, fin, fhid, fout):
    n_tiles2 = hp["n_tiles2"]
    npcp2 = hp["npcp2"]
    tw = hp["tw"]
    tw2 = tw + (tw & 1)
    x = np.asarray(inputs["x"], dtype=np.float32)

    consts = np.zeros((128, CW), dtype=NP_BF16)
    for name, c0 in (("W1_0", W10_C), ("W1_1", W11_C), ("Wx_0", WX0_C),
                     ("Wx_1", WX1_C), ("W2_0", W20_C), ("W2_1", W21_C)):
        w = np.asarray(inputs[name], np.float32).astype(NP_BF16)
        consts[0:w.shape[0], c0:c0 + w.shape[1]] = w
    consts[0:fhid, B1_C] = np.asarray(inputs["b1"], np.float32).astype(
        NP_BF16
    )
    consts[0:fhid, BX_C] = np.asarray(inputs["bx"], np.float32).astype(
        NP_BF16
    )
    consts[0:fout, B2_C] = np.asarray(inputs["b2"], np.float32).astype(
        NP_BF16
    )

    in_maps = []
    for c in range(M_CORES):
        xp = np.zeros((npcp2, 2 * XW), dtype=np.float32)
        xp[hp["new_of_old"][c], :fin] = x[c * npc:(c + 1) * npc]
        xtiles = np.ascontiguousarray(
            xp.reshape(n_tiles2, 128, 2 * XW).transpose(1, 0, 2).reshape(
                128, n_tiles2 * 2 * XW
            )
        ).astype(NP_BF16)
        wpad = np.zeros((128, tw2), dtype=NP_BF16)
        wpad[:, :tw] = hp["per_core"][c]["wgt"].astype(NP_BF16)
        payload = np.concatenate([xtiles, consts, wpad], axis=1)
        blob = np.concatenate(
            [hp["per_core"][c]["offs"], payload.view(np.int32)], axis=1
        )
        in_maps.append(dict(blob=np.ascontiguousarray(blob)))
    return in_maps


def run(inputs, n_nodes, fin, fhid, fout, trace=False, trace_kwargs=None,
        timeit=0):
    npc = n_nodes // M_CORES

    adj = np.asarray(inputs["adj"], dtype=np.int32)
    hp = host_prep(adj, n_nodes, npc)
    nc = build_program(hp, fin, fhid, fout)
    in_maps = make_in_maps(inputs, n_nodes, npc, hp, fin, fhid, fout)
    res = run_bass_kernel_spmd(
        nc,
        in_maps,
        core_ids=list(range(M_CORES)),
        trace=trace,
        **(trace_kwargs or {}),
    )
    times = []
    for _ in range(timeit):
        t0 = time.perf_counter()
        run_bass_kernel_spmd(nc, in_maps, core_ids=list(range(M_CORES)))
        times.append(time.perf_counter() - t0)
    if times:
        print("repeat wall times (s):", [f"{t:.3f}" for t in times])
        global LAST_TIMES
        LAST_TIMES = times
    out = np.concatenate(
        [
            np.asarray(res.results[c]["out"])[:, hp["new_of_old"][c]]
            .T.astype(np.float32)
            for c in range(M_CORES)
        ],
        axis=0,
    )
    return out, res


def kernel(**inputs):
    out, _ = run(inputs, n_nodes=100000, fin=64, fhid=64, fout=16)
    return out


# revision 15
# speedup vs baseline: 1.1095x; 1.1095x over previous
"""ChebGCN (K=2, 3 layers) Trainium2 kernel — 8-core SPMD.

Sharding: nodes are split across 8 cores (12500/core). Within a core,
local nodes are PERMUTED so they are grouped by degree bucket (host-side
renumbering; inputs/outputs are permuted on the host for free). Each
node's incoming edges occupy a fixed per-bucket budget of M slots
(M in {12,16,20,24,tail}, zero-padded), so the whole propagate becomes:
indirect-DMA gather of 128 source rows per (tile, occurrence) slot column,
one broadcast tensor_tensor to apply edge weights, and ONE tensor_reduce
over the innermost occurrence axis per tile group — no per-chunk matmuls.
Per layer the reduced segment sums are stored row-major by transposed-AP
DMA, re-transposed whole-table by one DMA-transpose for the dense term,
and the dense 64-wide weight matmuls + bias/relu run 4 node-tiles (512
cols) per op in transposed layout with running features resident in SBUF.
An 8-core AllGather rebuilds the global row table per layer.

Host->device traffic is one int32 tensor per core packing, bit-cast per
region: per-slot source indices (int32) and weights (bf16), the permuted
bf16 x slice, and weight/bias constants. Output leaves as bf16 and is
un-permuted on the host.
"""

import sys

for _p in ("/opt/trn_rl_repo",):
    if _p not in sys.path:
        sys.path.insert(0, _p)

import math
import time
from contextlib import ExitStack

import ml_dtypes
import numpy as np

import concourse.bacc as bacc
import concourse.bass as bass
import concourse.mybir as mybir
import concourse.tile as tile
from concourse.bass_utils import run_bass_kernel_spmd

F32 = mybir.dt.float32
I32 = mybir.dt.int32
I8 = mybir.dt.int8
BF16 = mybir.dt.bfloat16
NP_BF16 = ml_dtypes.bfloat16

M_CORES = 8
TG = 4        # node tiles per dense matmul batch (512 cols)
XW = 16       # int32 cols per x tile in the blob (64 int8 feats)
TMAX = 96     # max slot-columns (Tt*M) per gather/reduce group
CW = 292      # bf16 const-region cols (even)
W10_C, W11_C, WX0_C, WX1_C = 0, 64, 128, 192
W20_C, W21_C, B1_C, BX_C, B2_C = 256, 272, 288, 289, 290
LAST_TIMES = []  # wall times of repeat runs (filled by run(timeit=N))


# ---------------------------------------------------------------- host prep
def host_prep(adj, n_nodes, npc):
    """Degree-bucket nodes per core, build slot tables.

    Returns dict with: groups [(t0, Tt, M, col0)], tile_m, n_tiles2, tw,
    per_core [{offs [128,TW] i32, wgt [128,TW] f32}], new_of_old [8][npc].
    """
    row = adj[0].astype(np.int64)
    col = adj[1].astype(np.int64)
    deg = np.bincount(row, minlength=n_nodes).astype(np.int64)
    dis = np.where(deg > 0, 1.0 / np.sqrt(np.maximum(deg, 1)), 0.0).astype(
        np.float32
    )
    w_all = (-(dis[row] * dis[col])).astype(np.float32)

    maxdeg = int(deg.max())
    ms = [m for m in (12, 16, 20, 24) if m < maxdeg]
    ms.append(max(maxdeg, (ms[-1] + 4) if ms else 12))
    nb = len(ms)
    ms_arr = np.array(ms)

    def bucket_of(d):
        return np.searchsorted(ms_arr, np.maximum(d, 1))

    b_of = [bucket_of(deg[c * npc:(c + 1) * npc]) for c in range(M_CORES)]
    counts = np.zeros((M_CORES, nb), dtype=np.int64)
    for c in range(M_CORES):
        counts[c] = np.bincount(b_of[c], minlength=nb)
    nb_tiles = np.array(
        [int(math.ceil(counts[:, b].max() / 128.0)) for b in range(nb)]
    )
    n_tiles2 = int(nb_tiles.sum())
    npcp2 = 128 * n_tiles2

    tile_m = []
    for b in range(nb):
        tile_m += [ms[b]] * int(nb_tiles[b])
    col0 = np.concatenate([[0], np.cumsum(tile_m)[:-1]]).astype(np.int64)
    tw = int(np.sum(tile_m))

    groups = []
    t = 0
    while t < n_tiles2:
        m = tile_m[t]
        tt = max(1, TMAX // m)
        tt = min(tt, n_tiles2 - t)
        while tile_m[t + tt - 1] != m:
            tt -= 1
        groups.append((t, tt, m, int(col0[t])))
        t += tt

    base_b = 128 * np.concatenate([[0], np.cumsum(nb_tiles)[:-1]])
    new_of_old = []
    for c in range(M_CORES):
        order = np.argsort(b_of[c], kind="stable")
        noo = np.zeros(npc, dtype=np.int64)
        pos_in_b = np.zeros(nb, dtype=np.int64)
        sorted_b = b_of[c][order]
        # positions within each bucket follow sorted order
        starts = np.searchsorted(sorted_b, np.arange(nb))
        noo[order] = base_b[sorted_b] + (np.arange(npc) - starts[sorted_b])
        new_of_old.append(noo)

    # global padded source index per edge
    sc = col // npc
    sl = col % npc
    noo_all = np.stack(new_of_old)  # [8, npc]
    colp = sc * npcp2 + noo_all[sc, sl]

    per_core = []
    core_of = row // npc
    for c in range(M_CORES):
        sel = np.nonzero(core_of == c)[0]
        d_loc = row[sel] - c * npc
        q = new_of_old[c][d_loc]
        order = np.argsort(q, kind="stable")
        sel = sel[order]
        qs = q[order]
        # occurrence j within each destination's slot budget
        seg_start = np.searchsorted(qs, qs)
        j = np.arange(len(qs)) - seg_start
        t_of = qs // 128
        p_of = qs % 128
        cols = col0[t_of] + j
        offs = np.zeros((128, tw), dtype=np.int32)
        wgt = np.zeros((128, tw), dtype=np.float32)
        offs[p_of, cols] = colp[sel]
        wgt[p_of, cols] = w_all[sel]
        per_core.append(dict(offs=offs, wgt=wgt))

    return dict(groups=groups, tile_m=tile_m, n_tiles2=n_tiles2, tw=tw,
                per_core=per_core, new_of_old=new_of_old, npcp2=npcp2)


def blob_geom(tw, n_tiles2):
    tw2 = tw + (tw & 1)
    xb = tw                      # x region start (i32 cols)
    cb = xb + n_tiles2 * XW      # const region
    wb = cb + CW // 2            # weight region
    w32 = wb + tw2 // 2
    return xb, cb, wb, w32


# ------------------------------------------------------------- bass program
def build_program(hp, fin, fhid, fout, xscale):
    groups = hp["groups"]
    n_tiles2 = hp["n_tiles2"]
    tw = hp["tw"]
    npcp2 = hp["npcp2"]
    np_all = npcp2 * M_CORES
    tw2 = tw + (tw & 1)
    xb, cbase, wbase, w32 = blob_geom(tw, n_tiles2)

    nc = bacc.Bacc(
        "TRN2",
        target_bir_lowering=False,
        debug=False,
        enable_asserts=False,
        num_devices=M_CORES,
    )

    blob_d = nc.dram_tensor("blob", [128, w32], I32, kind="ExternalInput")
    out_d = nc.dram_tensor("out", [fout, npcp2], BF16, kind="ExternalOutput")

    xrows = nc.dram_tensor("xrows", [npcp2, fin], I8)
    x16_d = nc.dram_tensor("x16", [npcp2, fin], BF16)
    rows1 = nc.dram_tensor("rows1", [npcp2, fhid], BF16)
    rows2 = nc.dram_tensor("rows2", [npcp2, fhid], BF16)
    tx1_d = nc.dram_tensor("tx1", [npcp2, fhid], BF16)
    tx2_d = nc.dram_tensor("tx2", [npcp2, fhid], BF16)
    tx3_d = nc.dram_tensor("tx3", [npcp2, fhid], BF16)
    tab1 = nc.dram_tensor("tab1", [np_all, fin], I8, addr_space="Shared")
    tab2 = nc.dram_tensor("tab2", [np_all, fhid], BF16, addr_space="Shared")
    tab3 = nc.dram_tensor("tab3", [np_all, fhid], BF16, addr_space="Shared")

    rg = [list(range(M_CORES))]

    with ExitStack() as ctx:
        tc = ctx.enter_context(tile.TileContext(nc))
        const = ctx.enter_context(tc.tile_pool(name="const", bufs=1))
        rp = ctx.enter_context(tc.tile_pool(name="rp", bufs=2))
        rwp = ctx.enter_context(tc.tile_pool(name="rwp", bufs=2))
        txop = ctx.enter_context(tc.tile_pool(name="txop", bufs=2))
        otp = ctx.enter_context(tc.tile_pool(name="otp", bufs=2))
        psB = ctx.enter_context(tc.tile_pool(name="psB", bufs=2, space="PSUM"))

        # const region: one DMA, then slice views
        cb = const.tile([128, CW], BF16, tag="cb")
        nc.sync.dma_start(
            out=cb[:],
            in_=blob_d[:, cbase:cbase + CW // 2].bitcast(BF16),
        )
        w10_t = cb[0:fin, W10_C:W10_C + fhid]
        w11_t = cb[0:fin, W11_C:W11_C + fhid]
        wx0_t = cb[0:fhid, WX0_C:WX0_C + fhid]
        wx1_t = cb[0:fhid, WX1_C:WX1_C + fhid]
        w20_t = cb[0:fhid, W20_C:W20_C + fout]
        w21_t = cb[0:fhid, W21_C:W21_C + fout]
        b1_t = const.tile([fhid, 1], F32, tag="b1")
        nc.vector.tensor_copy(out=b1_t[:], in_=cb[0:fhid, B1_C:B1_C + 1])
        bx_t = const.tile([fhid, 1], F32, tag="bx")
        nc.vector.tensor_copy(out=bx_t[:], in_=cb[0:fhid, BX_C:BX_C + 1])
        b2_t = const.tile([fout, 1], F32, tag="b2")
        nc.vector.tensor_copy(out=b2_t[:], in_=cb[0:fout, B2_C:B2_C + 1])

        # slot metadata: two bulk DMAs, no unpacking needed
        offs_all = const.tile([128, tw], I32, tag="offs")
        nc.sync.dma_start(out=offs_all[:], in_=blob_d[:, 0:tw])
        w_sb = const.tile([128, tw2], BF16, tag="wal")
        nc.sync.dma_start(
            out=w_sb[:],
            in_=blob_d[:, wbase:wbase + tw2 // 2].bitcast(BF16),
        )

        # x prologue: bulk load, row-major store, AllGather, transpose
        xa = const.tile([128, n_tiles2 * XW], I32, tag="xa")
        nc.sync.dma_start(out=xa[:],
                          in_=blob_d[:, xb:xb + n_tiles2 * XW])
        xa8 = xa[:].bitcast(I8).rearrange("p (t f) -> p t f", f=4 * XW)
        nc.sync.dma_start(
            out=xrows[:, :].rearrange("(t p) f -> p t f", p=128),
            in_=xa8,
        )
        nc.gpsimd.collective_compute(
            "AllGather",
            mybir.AluOpType.bypass,
            replica_groups=rg,
            ins=[xrows[:, :]],
            outs=[tab1[:, :]],
        )
        # dequantized bf16 copy of x rows, only for the dense-term x^T
        xc = const.tile([128, n_tiles2, fin], BF16, tag="xc")
        nc.vector.tensor_scalar(
            out=xc[:], in0=xa8, scalar1=float(xscale), scalar2=None,
            op0=mybir.AluOpType.mult,
        )
        nc.sync.dma_start(
            out=x16_d[:, :].rearrange("(t p) f -> p t f", p=128),
            in_=xc[:],
        )
        xT_sb = const.tile([fin, npcp2], BF16, tag="xT")
        nc.sync.dma_start_transpose(xT_sb[:], x16_d[:, :])

        txT_sb = const.tile([fhid, npcp2], BF16, tag="txT")
        hT1_sb = const.tile([fhid, npcp2], BF16, tag="hT1")
        hT2_sb = const.tile([fhid, npcp2], BF16, tag="hT2")

        layers = [
            dict(table=tab1, rhs_sb=xT_sb, W0=w10_t, W1=w11_t, b=b1_t,
                 fo=fhid, hT_next=hT1_sb, rows=rows1, tab_next=tab2,
                 tx_d=tx1_d),
            dict(table=tab2, rhs_sb=hT1_sb, W0=wx0_t, W1=wx1_t, b=bx_t,
                 fo=fhid, hT_next=hT2_sb, rows=rows2, tab_next=tab3,
                 tx_d=tx2_d),
            dict(table=tab3, rhs_sb=hT2_sb, W0=w20_t, W1=w21_t, b=b2_t,
                 fo=fout, hT_next=None, rows=None, tab_next=None,
                 tx_d=tx3_d),
        ]

        for li, L in enumerate(layers):
            fo = L["fo"]
            rdt = I8 if li == 0 else BF16
            # propagate: gather / weight / reduce per tile group
            for (t0, tt, m, c0) in groups:
                r = rp.tile([128, tt, m, fin], rdt, tag="r")
                for t in range(tt):
                    for j in range(m):
                        nc.gpsimd.indirect_dma_start(
                            out=r[:, t, j, :],
                            out_offset=None,
                            in_=L["table"][:, :],
                            in_offset=bass.IndirectOffsetOnAxis(
                                ap=offs_all[:, c0 + t * m + j:
                                            c0 + t * m + j + 1],
                                axis=0,
                            ),
                        )
                rw = rwp.tile([128, tt, m, fin], BF16, tag="rw")
                nc.vector.tensor_tensor(
                    out=rw[:],
                    in0=r[:],
                    in1=w_sb[:, c0:c0 + tt * m].rearrange(
                        "p (t m) -> p t m", m=m
                    ).unsqueeze(3).broadcast_to([128, tt, m, fin]),
                    op=mybir.AluOpType.mult,
                )
                txo = txop.tile([128, tt, fin], BF16, tag="txo")
                with nc.allow_low_precision(
                    reason="segment sums of <=40 bf16 terms; tolerance 2e-2"
                ):
                    nc.vector.tensor_reduce(
                        out=txo[:],
                        in_=rw[:].rearrange("p t m f -> p t f m"),
                        axis=mybir.AxisListType.X,
                        op=mybir.AluOpType.add,
                    )
                nc.sync.dma_start(
                    out=L["tx_d"][t0 * 128:(t0 + tt) * 128, :].rearrange(
                        "(t p) f -> p t f", p=128
                    ),
                    in_=txo[:],
                )
            # whole-layer transpose of segment sums for the dense term
            nc.sync.dma_start_transpose(txT_sb[:], L["tx_d"][:, :])
            # dense + bias/relu, TG node tiles per op
            for q0 in range(0, n_tiles2, TG):
                qt = min(TG, n_tiles2 - q0)
                c0 = q0 * 128
                gw = qt * 128
                pb = psB.tile([fo, TG * 128], F32, tag="pb")
                nc.tensor.matmul(
                    pb[:, :gw], lhsT=L["W0"],
                    rhs=L["rhs_sb"][:, c0:c0 + gw],
                    start=True, stop=False,
                )
                nc.tensor.matmul(
                    pb[:, :gw], lhsT=L["W1"],
                    rhs=txT_sb[:, c0:c0 + gw],
                    start=False, stop=True,
                )
                if L["hT_next"] is not None:
                    osl = L["hT_next"][:, c0:c0 + gw]
                    nc.scalar.activation(
                        osl, pb[:, :gw],
                        mybir.ActivationFunctionType.Relu,
                        bias=L["b"][:],
                    )
                    nc.sync.dma_start(
                        out=L["rows"][c0:c0 + gw, :].rearrange(
                            "n f -> f n"
                        ),
                        in_=osl,
                    )
                else:
                    ot = otp.tile([fout, TG * 128], BF16, tag="ot")
                    nc.scalar.activation(
                        ot[:, :gw], pb[:, :gw],
                        mybir.ActivationFunctionType.Identity,
                        bias=L["b"][:],
                    )
                    nc.sync.dma_start(
                        out=out_d[:, c0:c0 + gw], in_=ot[:, :gw]
                    )
            if L["tab_next"] is not None:
                nc.gpsimd.collective_compute(
                    "AllGather",
                    mybir.AluOpType.bypass,
                    replica_groups=rg,
                    ins=[L["rows"][:, :]],
                    outs=[L["tab_next"][:, :]],
                )

    nc.compile()
    return nc


# ------------------------------------------------------------------ runner
def make_in_maps(inputs, n_nodes, npc, hp, fin, fhid, fout, xscale):
    n_tiles2 = hp["n_tiles2"]
    npcp2 = hp["npcp2"]
    tw = hp["tw"]
    tw2 = tw + (tw & 1)
    x = np.asarray(inputs["x"], dtype=np.float32)

    consts = np.zeros((128, CW), dtype=NP_BF16)
    for name, c0 in (("W1_0", W10_C), ("W1_1", W11_C), ("Wx_0", WX0_C),
                     ("Wx_1", WX1_C), ("W2_0", W20_C), ("W2_1", W21_C)):
        w = np.asarray(inputs[name], np.float32)
        if name == "W1_1":
            # gather path feeds integer-valued x/s; fold the scale here
            w = w * xscale
        w = w.astype(NP_BF16)
        consts[0:w.shape[0], c0:c0 + w.shape[1]] = w
    consts[0:fhid, B1_C] = np.asarray(inputs["b1"], np.float32).astype(
        NP_BF16
    )
    consts[0:fhid, BX_C] = np.asarray(inputs["bx"], np.float32).astype(
        NP_BF16
    )
    consts[0:fout, B2_C] = np.asarray(inputs["b2"], np.float32).astype(
        NP_BF16
    )

    in_maps = []
    for c in range(M_CORES):
        xp = np.zeros((npcp2, 4 * XW), dtype=np.float32)
        xp[hp["new_of_old"][c], :fin] = x[c * npc:(c + 1) * npc]
        xq = np.clip(np.round(xp / xscale), -127, 127).astype(np.int8)
        xtiles = np.ascontiguousarray(
            xq.reshape(n_tiles2, 128, 4 * XW).transpose(1, 0, 2).reshape(
                128, n_tiles2 * 4 * XW
            )
        )
        wpad = np.zeros((128, tw2), dtype=NP_BF16)
        wpad[:, :tw] = hp["per_core"][c]["wgt"].astype(NP_BF16)
        payload = np.concatenate(
            [xtiles.view(np.int32), consts.view(np.int32),
             wpad.view(np.int32)], axis=1
        )
        blob = np.concatenate([hp["per_core"][c]["offs"], payload], axis=1)
        in_maps.append(dict(blob=np.ascontiguousarray(blob)))
    return in_maps


def run(inputs, n_nodes, fin, fhid, fout, trace=False, trace_kwargs=None,
        timeit=0):
    npc = n_nodes // M_CORES

    adj = np.asarray(inputs["adj"], dtype=np.int32)
    hp = host_prep(adj, n_nodes, npc)
    xscale = float(np.abs(np.asarray(inputs["x"])).max() / 127.0)
    nc = build_program(hp, fin, fhid, fout, xscale)
    in_maps = make_in_maps(inputs, n_nodes, npc, hp, fin, fhid, fout,
                           xscale)
    res = run_bass_kernel_spmd(
        nc,
        in_maps,
        core_ids=list(range(M_CORES)),
        trace=trace,
        **(trace_kwargs or {}),
    )
    times = []
    for _ in range(timeit):
        t0 = time.perf_counter()
        run_bass_kernel_spmd(nc, in_maps, core_ids=list(range(M_CORES)))
        times.append(time.perf_counter() - t0)
    if times:
        print("repeat wall times (s):", [f"{t:.3f}" for t in times])
        global LAST_TIMES
        LAST_TIMES = times
    out = np.concatenate(
        [
            np.asarray(res.results[c]["out"])[:, hp["new_of_old"][c]]
            .T.astype(np.float32)
            for c in range(M_CORES)
        ],
        axis=0,
    )
    return out, res


def kernel(**inputs):
    out, _ = run(inputs, n_nodes=100000, fin=64, fhid=64, fout=16)
    return out


# revision 17
# speedup vs baseline: 1.3532x; 1.2196x over previous
"""ChebGCN (K=2, 3 layers) Trainium2 kernel — 8-core SPMD.

Sharding: nodes are split across 8 cores (12500/core). Within a core,
local nodes are PERMUTED so they are grouped by degree bucket (host-side
renumbering; inputs/outputs are permuted on the host for free). Each
node's incoming edges occupy a fixed per-bucket budget of M slots
(M in {12,16,20,24,tail}, zero-padded), so the whole propagate becomes:
indirect-DMA gather of 128 source rows per (tile, occurrence) slot column,
one broadcast tensor_tensor to apply edge weights, and ONE tensor_reduce
over the innermost occurrence axis per tile group — no per-chunk matmuls.
Per layer the reduced segment sums are stored row-major by transposed-AP
DMA, re-transposed whole-table by one DMA-transpose for the dense term,
and the dense 64-wide weight matmuls + bias/relu run 4 node-tiles (512
cols) per op in transposed layout with running features resident in SBUF.
An 8-core AllGather rebuilds the global row table per layer.

Host->device traffic is one int32 tensor per core packing, bit-cast per
region: per-slot source indices (int32) and weights (bf16), the permuted
x slice quantized to int8 (global scale; the gather path needs no device
dequant because the scale is folded into W1_1 on the host, and only the
dense-term x^T copy is dequantized on device), and weight/bias constants.
Output leaves as bf16 and is un-permuted on the host.
"""

import sys

for _p in ("/opt/trn_rl_repo",):
    if _p not in sys.path:
        sys.path.insert(0, _p)

import math
import time
from contextlib import ExitStack

import ml_dtypes
import numpy as np

import concourse.bacc as bacc
import concourse.bass as bass
import concourse.mybir as mybir
import concourse.tile as tile
from concourse.bass_utils import run_bass_kernel_spmd

F32 = mybir.dt.float32
I32 = mybir.dt.int32
I8 = mybir.dt.int8
U16 = mybir.dt.uint16
BF16 = mybir.dt.bfloat16
NP_BF16 = ml_dtypes.bfloat16

M_CORES = 8
TG = 4        # node tiles per dense matmul batch (512 cols)
XW = 16       # int32 cols per x tile in the blob (64 int8 feats)
TMAX = 96     # max slot-columns (Tt*M) per gather/reduce group
CW = 292      # bf16 const-region cols (even)
W10_C, W11_C, WX0_C, WX1_C = 0, 64, 128, 192
W20_C, W21_C, B1_C, BX_C, B2_C = 256, 272, 288, 289, 290
LAST_TIMES = []  # wall times of repeat runs (filled by run(timeit=N))


# ---------------------------------------------------------------- host prep
def host_prep(adj, n_nodes, npc):
    """Degree-bucket nodes per core, build slot tables.

    Returns dict with: groups [(t0, Tt, M, col0)], tile_m, n_tiles2, tw,
    per_core [{offs [128,TW] i32, wgt [128,TW] f32}], new_of_old [8][npc].
    """
    row = adj[0].astype(np.int64)
    col = adj[1].astype(np.int64)
    deg = np.bincount(row, minlength=n_nodes).astype(np.int64)
    dis = np.where(deg > 0, 1.0 / np.sqrt(np.maximum(deg, 1)), 0.0).astype(
        np.float32
    )
    w_all = (-(dis[row] * dis[col])).astype(np.float32)

    n_tiles2 = int(math.ceil(npc / 128.0))
    npcp2 = 128 * n_tiles2

    # degree-sorted local order; per-tile slot budget = cross-core max
    # degree within the tile (sorted tiles have near-uniform degree)
    new_of_old = []
    raw_m = np.zeros((M_CORES, n_tiles2), dtype=np.int64)
    for c in range(M_CORES):
        degl = deg[c * npc:(c + 1) * npc]
        order = np.argsort(degl, kind="stable")
        noo = np.zeros(npc, dtype=np.int64)
        noo[order] = np.arange(npc)
        new_of_old.append(noo)
        dpad = np.zeros(npcp2, dtype=np.int64)
        dpad[:npc] = degl[order]
        raw_m[c] = dpad.reshape(n_tiles2, 128).max(axis=1)
    tile_raw = np.maximum(raw_m.max(axis=0), 1)

    # greedy grouping of consecutive tiles; group slot budget = window max
    groups = []
    tile_m = np.zeros(n_tiles2, dtype=np.int64)
    t = 0
    while t < n_tiles2:
        tt = 1
        m = int(tile_raw[t])
        while t + tt < n_tiles2 and (tt + 1) * max(
            m, int(tile_raw[t + tt])
        ) <= TMAX:
            m = max(m, int(tile_raw[t + tt]))
            tt += 1
        tile_m[t:t + tt] = m
        groups.append((t, tt, m, 0))
        t += tt
    col0 = np.concatenate([[0], np.cumsum(tile_m)[:-1]]).astype(np.int64)
    tw = int(np.sum(tile_m))
    groups = [(t0, tt, m, int(col0[t0])) for (t0, tt, m, _) in groups]

    # global padded source index per edge
    sc = col // npc
    sl = col % npc
    noo_all = np.stack(new_of_old)  # [8, npc]
    colp = sc * npcp2 + noo_all[sc, sl]

    per_core = []
    core_of = row // npc
    for c in range(M_CORES):
        sel = np.nonzero(core_of == c)[0]
        d_loc = row[sel] - c * npc
        q = new_of_old[c][d_loc]
        order = np.argsort(q, kind="stable")
        sel = sel[order]
        qs = q[order]
        # occurrence j within each destination's slot budget
        seg_start = np.searchsorted(qs, qs)
        j = np.arange(len(qs)) - seg_start
        t_of = qs // 128
        p_of = qs % 128
        cols = col0[t_of] + j
        offs = np.zeros((128, tw), dtype=np.uint32)
        wbits = np.zeros((128, tw), dtype=np.uint32)
        offs[p_of, cols] = colp[sel].astype(np.uint32)
        # |w| <= 1 so the bf16 sign bit is 0 after abs and the exponent
        # MSB is 0: 15 bits are lossless and the packed word stays >= 0
        wb = np.abs(w_all[sel]).astype(NP_BF16).view(np.uint16)
        wbits[p_of, cols] = wb.astype(np.uint32)
        packed = ((wbits << 17) | offs).view(np.int32)
        per_core.append(dict(packed=packed))

    return dict(groups=groups, tile_m=tile_m, n_tiles2=n_tiles2, tw=tw,
                per_core=per_core, new_of_old=new_of_old, npcp2=npcp2)


def blob_geom(tw, n_tiles2):
    xb = tw                      # x region start (i32 cols)
    cb = xb + n_tiles2 * XW      # const region
    w32 = cb + CW // 2
    return xb, cb, w32


# ------------------------------------------------------------- bass program
def build_program(hp, fin, fhid, fout, xscale):
    groups = hp["groups"]
    n_tiles2 = hp["n_tiles2"]
    tw = hp["tw"]
    npcp2 = hp["npcp2"]
    np_all = npcp2 * M_CORES
    xb, cbase, w32 = blob_geom(tw, n_tiles2)

    nc = bacc.Bacc(
        "TRN2",
        target_bir_lowering=False,
        debug=False,
        enable_asserts=False,
        num_devices=M_CORES,
    )

    blob_d = nc.dram_tensor("blob", [128, w32], I32, kind="ExternalInput")
    out_d = nc.dram_tensor("out", [fout, npcp2], BF16, kind="ExternalOutput")

    xrows = nc.dram_tensor("xrows", [npcp2, fin], I8)
    x16_d = nc.dram_tensor("x16", [npcp2, fin], BF16)
    rows1 = nc.dram_tensor("rows1", [npcp2, fhid], BF16)
    rows2 = nc.dram_tensor("rows2", [npcp2, fhid], BF16)
    tx1_d = nc.dram_tensor("tx1", [npcp2, fhid], BF16)
    tx2_d = nc.dram_tensor("tx2", [npcp2, fhid], BF16)
    tx3_d = nc.dram_tensor("tx3", [npcp2, fhid], BF16)
    tab1 = nc.dram_tensor("tab1", [np_all, fin], I8, addr_space="Shared")
    tab2 = nc.dram_tensor("tab2", [np_all, fhid], BF16, addr_space="Shared")
    tab3 = nc.dram_tensor("tab3", [np_all, fhid], BF16, addr_space="Shared")

    rg = [list(range(M_CORES))]

    with ExitStack() as ctx:
        tc = ctx.enter_context(tile.TileContext(nc))
        const = ctx.enter_context(tc.tile_pool(name="const", bufs=1))
        rp = ctx.enter_context(tc.tile_pool(name="rp", bufs=2))
        rwp = ctx.enter_context(tc.tile_pool(name="rwp", bufs=2))
        txop = ctx.enter_context(tc.tile_pool(name="txop", bufs=2))
        otp = ctx.enter_context(tc.tile_pool(name="otp", bufs=2))
        psB = ctx.enter_context(tc.tile_pool(name="psB", bufs=2, space="PSUM"))

        # const region: one DMA, then slice views
        cb = const.tile([128, CW], BF16, tag="cb")
        nc.sync.dma_start(
            out=cb[:],
            in_=blob_d[:, cbase:cbase + CW // 2].bitcast(BF16),
        )
        w10_t = cb[0:fin, W10_C:W10_C + fhid]
        w11_t = cb[0:fin, W11_C:W11_C + fhid]
        wx0_t = cb[0:fhid, WX0_C:WX0_C + fhid]
        wx1_t = cb[0:fhid, WX1_C:WX1_C + fhid]
        w20_t = cb[0:fhid, W20_C:W20_C + fout]
        w21_t = cb[0:fhid, W21_C:W21_C + fout]
        b1_t = const.tile([fhid, 1], F32, tag="b1")
        nc.vector.tensor_copy(out=b1_t[:], in_=cb[0:fhid, B1_C:B1_C + 1])
        bx_t = const.tile([fhid, 1], F32, tag="bx")
        nc.vector.tensor_copy(out=bx_t[:], in_=cb[0:fhid, BX_C:BX_C + 1])
        b2_t = const.tile([fout, 1], F32, tag="b2")
        nc.vector.tensor_copy(out=b2_t[:], in_=cb[0:fout, B2_C:B2_C + 1])

        # slot metadata: one bulk DMA + 3-op unpack of (wbits15<<17|offs)
        pk = const.tile([128, tw], I32, tag="pk")
        nc.sync.dma_start(out=pk[:], in_=blob_d[:, 0:tw])
        offs_all = const.tile([128, tw], I32, tag="offs")
        nc.vector.tensor_scalar(
            out=offs_all[:], in0=pk[:], scalar1=131071, scalar2=None,
            op0=mybir.AluOpType.bitwise_and,
        )
        wn_i = const.tile([128, tw], I32, tag="wni")
        nc.vector.tensor_scalar(
            out=wn_i[:], in0=pk[:], scalar1=17, scalar2=32768,
            op0=mybir.AluOpType.logical_shift_right,
            op1=mybir.AluOpType.bitwise_or,
        )
        w_u16 = const.tile([128, tw], U16, tag="wal")
        nc.vector.tensor_copy(out=w_u16[:], in_=wn_i[:])

        # x prologue: bulk load, row-major store, AllGather, transpose
        xa = const.tile([128, n_tiles2 * XW], I32, tag="xa")
        nc.sync.dma_start(out=xa[:],
                          in_=blob_d[:, xb:xb + n_tiles2 * XW])
        xa8 = xa[:].bitcast(I8).rearrange("p (t f) -> p t f", f=4 * XW)
        nc.sync.dma_start(
            out=xrows[:, :].rearrange("(t p) f -> p t f", p=128),
            in_=xa8,
        )
        nc.gpsimd.collective_compute(
            "AllGather",
            mybir.AluOpType.bypass,
            replica_groups=rg,
            ins=[xrows[:, :]],
            outs=[tab1[:, :]],
        )
        # dequantized bf16 copy of x rows, only for the dense-term x^T
        xc = const.tile([128, n_tiles2, fin], BF16, tag="xc")
        nc.vector.tensor_scalar(
            out=xc[:], in0=xa8, scalar1=float(xscale), scalar2=None,
            op0=mybir.AluOpType.mult,
        )
        nc.sync.dma_start(
            out=x16_d[:, :].rearrange("(t p) f -> p t f", p=128),
            in_=xc[:],
        )
        xT_sb = const.tile([fin, npcp2], BF16, tag="xT")
        nc.sync.dma_start_transpose(xT_sb[:], x16_d[:, :])

        txT_sb = const.tile([fhid, npcp2], BF16, tag="txT")
        hT1_sb = const.tile([fhid, npcp2], BF16, tag="hT1")
        hT2_sb = const.tile([fhid, npcp2], BF16, tag="hT2")

        layers = [
            dict(table=tab1, rhs_sb=xT_sb, W0=w10_t, W1=w11_t, b=b1_t,
                 fo=fhid, hT_next=hT1_sb, rows=rows1, tab_next=tab2,
                 tx_d=tx1_d),
            dict(table=tab2, rhs_sb=hT1_sb, W0=wx0_t, W1=wx1_t, b=bx_t,
                 fo=fhid, hT_next=hT2_sb, rows=rows2, tab_next=tab3,
                 tx_d=tx2_d),
            dict(table=tab3, rhs_sb=hT2_sb, W0=w20_t, W1=w21_t, b=b2_t,
                 fo=fout, hT_next=None, rows=None, tab_next=None,
                 tx_d=tx3_d),
        ]

        for li, L in enumerate(layers):
            fo = L["fo"]
            rdt = I8 if li == 0 else BF16
            # propagate: gather / weight / reduce per tile group
            for (t0, tt, m, c0) in groups:
                r = rp.tile([128, tt, m, fin], rdt, tag="r")
                for t in range(tt):
                    for j in range(m):
                        nc.gpsimd.indirect_dma_start(
                            out=r[:, t, j, :],
                            out_offset=None,
                            in_=L["table"][:, :],
                            in_offset=bass.IndirectOffsetOnAxis(
                                ap=offs_all[:, c0 + t * m + j:
                                            c0 + t * m + j + 1],
                                axis=0,
                            ),
                        )
                rw = rwp.tile([128, tt, m, fin], BF16, tag="rw")
                nc.vector.tensor_tensor(
                    out=rw[:],
                    in0=r[:],
                    in1=w_u16[:, c0:c0 + tt * m].bitcast(BF16).rearrange(
                        "p (t m) -> p t m", m=m
                    ).unsqueeze(3).broadcast_to([128, tt, m, fin]),
                    op=mybir.AluOpType.mult,
                )
                txo = txop.tile([128, tt, fin], BF16, tag="txo")
                with nc.allow_low_precision(
                    reason="segment sums of <=40 bf16 terms; tolerance 2e-2"
                ):
                    nc.vector.tensor_reduce(
                        out=txo[:],
                        in_=rw[:].rearrange("p t m f -> p t f m"),
                        axis=mybir.AxisListType.X,
                        op=mybir.AluOpType.add,
                    )
                nc.sync.dma_start(
                    out=L["tx_d"][t0 * 128:(t0 + tt) * 128, :].rearrange(
                        "(t p) f -> p t f", p=128
                    ),
                    in_=txo[:],
                )
            # whole-layer transpose of segment sums for the dense term
            nc.sync.dma_start_transpose(txT_sb[:], L["tx_d"][:, :])
            # dense + bias/relu, TG node tiles per op
            for q0 in range(0, n_tiles2, TG):
                qt = min(TG, n_tiles2 - q0)
                c0 = q0 * 128
                gw = qt * 128
                pb = psB.tile([fo, TG * 128], F32, tag="pb")
                nc.tensor.matmul(
                    pb[:, :gw], lhsT=L["W0"],
                    rhs=L["rhs_sb"][:, c0:c0 + gw],
                    start=True, stop=False,
                )
                nc.tensor.matmul(
                    pb[:, :gw], lhsT=L["W1"],
                    rhs=txT_sb[:, c0:c0 + gw],
                    start=False, stop=True,
                )
                if L["hT_next"] is not None:
                    osl = L["hT_next"][:, c0:c0 + gw]
                    nc.scalar.activation(
                        osl, pb[:, :gw],
                        mybir.ActivationFunctionType.Relu,
                        bias=L["b"][:],
                    )
                    nc.sync.dma_start(
                        out=L["rows"][c0:c0 + gw, :].rearrange(
                            "n f -> f n"
                        ),
                        in_=osl,
                    )
                else:
                    ot = otp.tile([fout, TG * 128], BF16, tag="ot")
                    nc.scalar.activation(
                        ot[:, :gw], pb[:, :gw],
                        mybir.ActivationFunctionType.Identity,
                        bias=L["b"][:],
                    )
                    nc.sync.dma_start(
                        out=out_d[:, c0:c0 + gw], in_=ot[:, :gw]
                    )
            if L["tab_next"] is not None:
                nc.gpsimd.collective_compute(
                    "AllGather",
                    mybir.AluOpType.bypass,
                    replica_groups=rg,
                    ins=[L["rows"][:, :]],
                    outs=[L["tab_next"][:, :]],
                )

    nc.compile()
    return nc


# ------------------------------------------------------------------ runner
def make_in_maps(inputs, n_nodes, npc, hp, fin, fhid, fout, xscale):
    n_tiles2 = hp["n_tiles2"]
    npcp2 = hp["npcp2"]
    x = np.asarray(inputs["x"], dtype=np.float32)

    consts = np.zeros((128, CW), dtype=NP_BF16)
    for name, c0 in (("W1_0", W10_C), ("W1_1", W11_C), ("Wx_0", WX0_C),
                     ("Wx_1", WX1_C), ("W2_0", W20_C), ("W2_1", W21_C)):
        w = np.asarray(inputs[name], np.float32)
        if name == "W1_1":
            # gather path feeds integer-valued x/s; fold the scale here
            w = w * xscale
        w = w.astype(NP_BF16)
        consts[0:w.shape[0], c0:c0 + w.shape[1]] = w
    consts[0:fhid, B1_C] = np.asarray(inputs["b1"], np.float32).astype(
        NP_BF16
    )
    consts[0:fhid, BX_C] = np.asarray(inputs["bx"], np.float32).astype(
        NP_BF16
    )
    consts[0:fout, B2_C] = np.asarray(inputs["b2"], np.float32).astype(
        NP_BF16
    )

    in_maps = []
    for c in range(M_CORES):
        xp = np.zeros((npcp2, 4 * XW), dtype=np.float32)
        xp[hp["new_of_old"][c], :fin] = x[c * npc:(c + 1) * npc]
        xq = np.clip(np.round(xp / xscale), -127, 127).astype(np.int8)
        xtiles = np.ascontiguousarray(
            xq.reshape(n_tiles2, 128, 4 * XW).transpose(1, 0, 2).reshape(
                128, n_tiles2 * 4 * XW
            )
        )
        payload = np.concatenate(
            [xtiles.view(np.int32), consts.view(np.int32)], axis=1
        )
        blob = np.concatenate([hp["per_core"][c]["packed"], payload],
                              axis=1)
        in_maps.append(dict(blob=np.ascontiguousarray(blob)))
    return in_maps


def run(inputs, n_nodes, fin, fhid, fout, trace=False, trace_kwargs=None,
        timeit=0):
    npc = n_nodes // M_CORES

    adj = np.asarray(inputs["adj"], dtype=np.int32)
    hp = host_prep(adj, n_nodes, npc)
    xscale = float(np.abs(np.asarray(inputs["x"])).max() / 127.0)
    nc = build_program(hp, fin, fhid, fout, xscale)
    in_maps = make_in_maps(inputs, n_nodes, npc, hp, fin, fhid, fout,
                           xscale)
    res = run_bass_kernel_spmd(
        nc,
        in_maps,
        core_ids=list(range(M_CORES)),
        trace=trace,
        **(trace_kwargs or {}),
    )
    times = []
    for _ in range(timeit):
        t0 = time.perf_counter()
        run_bass_kernel_spmd(nc, in_maps, core_ids=list(range(M_CORES)))
        times.append(time.perf_counter() - t0)
    if times:
        print("repeat wall times (s):", [f"{t:.3f}" for t in times])
        global LAST_TIMES
        LAST_TIMES = times
    out = np.concatenate(
        [
            np.asarray(res.results[c]["out"])[:, hp["new_of_old"][c]]
            .T.astype(np.float32)
            for c in range(M_CORES)
        ],
        axis=0,
    )
    return out, res


def kernel(**inputs):
    out, _ = run(inputs, n_nodes=100000, fin=64, fhid=64, fout=16)
    return out


# revision 19
# speedup vs baseline: 1.3604x; 1.0053x over previous
"""ChebGCN (K=2, 3 layers) Trainium2 kernel — 8-core SPMD.

Sharding: nodes are split across 8 cores (12500/core). Within a core,
local nodes are PERMUTED into degree-sorted order (host-side renumbering;
inputs/outputs are permuted on the host for free). Each node's incoming
edges occupy a per-tile slot budget M = cross-core max degree within its
128-node tile (sorted tiles are near-uniform in degree, so zero-padding
is only a few percent), so the whole propagate becomes:
indirect-DMA gather of 128 source rows per (tile, occurrence) slot column,
one broadcast tensor_tensor to apply edge weights, and ONE tensor_reduce
over the innermost occurrence axis per tile group — no per-chunk matmuls.
Per layer the reduced segment sums are stored row-major by transposed-AP
DMA, re-transposed whole-table by one DMA-transpose for the dense term,
and the dense 64-wide weight matmuls + bias/relu run 4 node-tiles (512
cols) per op in transposed layout with running features resident in SBUF.
An 8-core AllGather rebuilds the global row table per layer.

Host->device traffic is one int32 tensor per core packing, bit-cast per
region: ONE int32 word per edge slot ((|w| bf16 bits15 << 17) | source
index — |w| <= 1 makes 15 bits lossless and keeps the word positive;
unpacked in 3 prologue ops), the permuted x slice quantized to int8
(global scale; the gather path needs no device dequant because the scale
is folded into W1_1 on the host, and only the dense-term x^T copy is
dequantized on device), and weight/bias constants. Output leaves as bf16
and is un-permuted on the host.
"""

import sys

for _p in ("/opt/trn_rl_repo",):
    if _p not in sys.path:
        sys.path.insert(0, _p)

import math
import time
from contextlib import ExitStack

import ml_dtypes
import numpy as np

import concourse.bacc as bacc
import concourse.bass as bass
import concourse.mybir as mybir
import concourse.tile as tile
from concourse.bass_utils import run_bass_kernel_spmd

F32 = mybir.dt.float32
I32 = mybir.dt.int32
I8 = mybir.dt.int8
U16 = mybir.dt.uint16
BF16 = mybir.dt.bfloat16
NP_BF16 = ml_dtypes.bfloat16

M_CORES = 8
TG = 4        # node tiles per dense matmul batch (512 cols)
XW = 16       # int32 cols per x tile in the blob (64 int8 feats)
TMAX = 96     # max slot-columns (Tt*M) per gather/reduce group
CW = 292      # bf16 const-region cols (even)
W10_C, W11_C, WX0_C, WX1_C = 0, 64, 128, 192
W20_C, W21_C, B1_C, BX_C, B2_C = 256, 272, 288, 289, 290
LAST_TIMES = []  # wall times of repeat runs (filled by run(timeit=N))


# ---------------------------------------------------------------- host prep
def host_prep(adj, n_nodes, npc):
    """Degree-bucket nodes per core, build slot tables.

    Returns dict with: groups [(t0, Tt, M, col0)], tile_m, n_tiles2, tw,
    per_core [{offs [128,TW] i32, wgt [128,TW] f32}], new_of_old [8][npc].
    """
    row = adj[0].astype(np.int64)
    col = adj[1].astype(np.int64)
    deg = np.bincount(row, minlength=n_nodes).astype(np.int64)
    dis = np.where(deg > 0, 1.0 / np.sqrt(np.maximum(deg, 1)), 0.0).astype(
        np.float32
    )
    w_all = (-(dis[row] * dis[col])).astype(np.float32)

    n_tiles2 = int(math.ceil(npc / 128.0))
    npcp2 = 128 * n_tiles2

    # degree-sorted local order; per-tile slot budget = cross-core max
    # degree within the tile (sorted tiles have near-uniform degree)
    new_of_old = []
    raw_m = np.zeros((M_CORES, n_tiles2), dtype=np.int64)
    for c in range(M_CORES):
        degl = deg[c * npc:(c + 1) * npc]
        order = np.argsort(degl, kind="stable")
        noo = np.zeros(npc, dtype=np.int64)
        noo[order] = np.arange(npc)
        new_of_old.append(noo)
        dpad = np.zeros(npcp2, dtype=np.int64)
        dpad[:npc] = degl[order]
        raw_m[c] = dpad.reshape(n_tiles2, 128).max(axis=1)
    tile_raw = np.maximum(raw_m.max(axis=0), 1)

    # greedy grouping of consecutive tiles; group slot budget = window max
    groups = []
    tile_m = np.zeros(n_tiles2, dtype=np.int64)
    t = 0
    while t < n_tiles2:
        tt = 1
        m = int(tile_raw[t])
        while t + tt < n_tiles2 and (tt + 1) * max(
            m, int(tile_raw[t + tt])
        ) <= TMAX:
            m = max(m, int(tile_raw[t + tt]))
            tt += 1
        tile_m[t:t + tt] = m
        groups.append((t, tt, m, 0))
        t += tt
    col0 = np.concatenate([[0], np.cumsum(tile_m)[:-1]]).astype(np.int64)
    tw = int(np.sum(tile_m))
    groups = [(t0, tt, m, int(col0[t0])) for (t0, tt, m, _) in groups]

    # global padded source index per edge
    sc = col // npc
    sl = col % npc
    noo_all = np.stack(new_of_old)  # [8, npc]
    colp = sc * npcp2 + noo_all[sc, sl]

    per_core = []
    core_of = row // npc
    for c in range(M_CORES):
        sel = np.nonzero(core_of == c)[0]
        d_loc = row[sel] - c * npc
        q = new_of_old[c][d_loc]
        order = np.argsort(q, kind="stable")
        sel = sel[order]
        qs = q[order]
        # occurrence j within each destination's slot budget
        seg_start = np.searchsorted(qs, qs)
        j = np.arange(len(qs)) - seg_start
        t_of = qs // 128
        p_of = qs % 128
        cols = col0[t_of] + j
        offs = np.zeros((128, tw), dtype=np.uint32)
        wbits = np.zeros((128, tw), dtype=np.uint32)
        offs[p_of, cols] = colp[sel].astype(np.uint32)
        # |w| <= 1 so the bf16 sign bit is 0 after abs and the exponent
        # MSB is 0: 15 bits are lossless and the packed word stays >= 0
        wb = np.abs(w_all[sel]).astype(NP_BF16).view(np.uint16)
        wbits[p_of, cols] = wb.astype(np.uint32)
        packed = ((wbits << 17) | offs).view(np.int32)
        per_core.append(dict(packed=packed))

    return dict(groups=groups, tile_m=tile_m, n_tiles2=n_tiles2, tw=tw,
                per_core=per_core, new_of_old=new_of_old, npcp2=npcp2)


def blob_geom(tw, n_tiles2):
    xb = tw                      # x region start (i32 cols)
    cb = xb + n_tiles2 * XW      # const region
    w32 = cb + CW // 2
    return xb, cb, w32


# ------------------------------------------------------------- bass program
def build_program(hp, fin, fhid, fout, xscale):
    groups = hp["groups"]
    n_tiles2 = hp["n_tiles2"]
    tw = hp["tw"]
    npcp2 = hp["npcp2"]
    np_all = npcp2 * M_CORES
    xb, cbase, w32 = blob_geom(tw, n_tiles2)

    nc = bacc.Bacc(
        "TRN2",
        target_bir_lowering=False,
        debug=False,
        enable_asserts=False,
        num_devices=M_CORES,
    )

    blob_d = nc.dram_tensor("blob", [128, w32], I32, kind="ExternalInput")
    out_d = nc.dram_tensor("out", [fout, npcp2], BF16, kind="ExternalOutput")

    xrows = nc.dram_tensor("xrows", [npcp2, fin], I8)
    x16_d = nc.dram_tensor("x16", [npcp2, fin], BF16)
    rows1 = nc.dram_tensor("rows1", [npcp2, fhid], BF16)
    rows2 = nc.dram_tensor("rows2", [npcp2, fhid], BF16)
    tx1_d = nc.dram_tensor("tx1", [npcp2, fhid], BF16)
    tx2_d = nc.dram_tensor("tx2", [npcp2, fhid], BF16)
    tx3_d = nc.dram_tensor("tx3", [npcp2, fhid], BF16)
    tab1 = nc.dram_tensor("tab1", [np_all, fin], I8, addr_space="Shared")
    tab2 = nc.dram_tensor("tab2", [np_all, fhid], BF16, addr_space="Shared")
    tab3 = nc.dram_tensor("tab3", [np_all, fhid], BF16, addr_space="Shared")

    rg = [list(range(M_CORES))]

    with ExitStack() as ctx:
        tc = ctx.enter_context(tile.TileContext(nc))
        const = ctx.enter_context(tc.tile_pool(name="const", bufs=1))
        rp = ctx.enter_context(tc.tile_pool(name="rp", bufs=2))
        rwp = ctx.enter_context(tc.tile_pool(name="rwp", bufs=2))
        txop = ctx.enter_context(tc.tile_pool(name="txop", bufs=2))
        otp = ctx.enter_context(tc.tile_pool(name="otp", bufs=2))
        psB = ctx.enter_context(tc.tile_pool(name="psB", bufs=2, space="PSUM"))

        # const region: one DMA, then slice views
        cb = const.tile([128, CW], BF16, tag="cb")
        nc.sync.dma_start(
            out=cb[:],
            in_=blob_d[:, cbase:cbase + CW // 2].bitcast(BF16),
        )
        w10_t = cb[0:fin, W10_C:W10_C + fhid]
        w11_t = cb[0:fin, W11_C:W11_C + fhid]
        wx0_t = cb[0:fhid, WX0_C:WX0_C + fhid]
        wx1_t = cb[0:fhid, WX1_C:WX1_C + fhid]
        w20_t = cb[0:fhid, W20_C:W20_C + fout]
        w21_t = cb[0:fhid, W21_C:W21_C + fout]
        b1_t = const.tile([fhid, 1], F32, tag="b1")
        nc.vector.tensor_copy(out=b1_t[:], in_=cb[0:fhid, B1_C:B1_C + 1])
        bx_t = const.tile([fhid, 1], F32, tag="bx")
        nc.vector.tensor_copy(out=bx_t[:], in_=cb[0:fhid, BX_C:BX_C + 1])
        b2_t = const.tile([fout, 1], F32, tag="b2")
        nc.vector.tensor_copy(out=b2_t[:], in_=cb[0:fout, B2_C:B2_C + 1])

        # slot metadata: one bulk DMA + 3-op unpack of (wbits15<<17|offs)
        pk = const.tile([128, tw], I32, tag="pk")
        nc.sync.dma_start(out=pk[:], in_=blob_d[:, 0:tw])
        offs_all = const.tile([128, tw], I32, tag="offs")
        nc.vector.tensor_scalar(
            out=offs_all[:], in0=pk[:], scalar1=131071, scalar2=None,
            op0=mybir.AluOpType.bitwise_and,
        )
        wn_i = const.tile([128, tw], I32, tag="wni")
        nc.vector.tensor_scalar(
            out=wn_i[:], in0=pk[:], scalar1=17, scalar2=32768,
            op0=mybir.AluOpType.logical_shift_right,
            op1=mybir.AluOpType.bitwise_or,
        )
        w_u16 = const.tile([128, tw], U16, tag="wal")
        nc.vector.tensor_copy(out=w_u16[:], in_=wn_i[:])

        # x prologue: bulk load, row-major store, AllGather, transpose
        xa = const.tile([128, n_tiles2 * XW], I32, tag="xa")
        nc.sync.dma_start(out=xa[:],
                          in_=blob_d[:, xb:xb + n_tiles2 * XW])
        xa8 = xa[:].bitcast(I8).rearrange("p (t f) -> p t f", f=4 * XW)
        nc.sync.dma_start(
            out=xrows[:, :].rearrange("(t p) f -> p t f", p=128),
            in_=xa8,
        )
        nc.gpsimd.collective_compute(
            "AllGather",
            mybir.AluOpType.bypass,
            replica_groups=rg,
            ins=[xrows[:, :]],
            outs=[tab1[:, :]],
        )
        # dequantized bf16 copy of x rows, only for the dense-term x^T
        xc = const.tile([128, n_tiles2, fin], BF16, tag="xc")
        nc.vector.tensor_scalar(
            out=xc[:], in0=xa8, scalar1=float(xscale), scalar2=None,
            op0=mybir.AluOpType.mult,
        )
        nc.sync.dma_start(
            out=x16_d[:, :].rearrange("(t p) f -> p t f", p=128),
            in_=xc[:],
        )
        xT_sb = const.tile([fin, npcp2], BF16, tag="xT")
        nc.sync.dma_start_transpose(xT_sb[:], x16_d[:, :])

        txT_sb = const.tile([fhid, npcp2], BF16, tag="txT")
        hT1_sb = const.tile([fhid, npcp2], BF16, tag="hT1")
        hT2_sb = const.tile([fhid, npcp2], BF16, tag="hT2")

        layers = [
            dict(table=tab1, rhs_sb=xT_sb, W0=w10_t, W1=w11_t, b=b1_t,
                 fo=fhid, hT_next=hT1_sb, rows=rows1, tab_next=tab2,
                 tx_d=tx1_d),
            dict(table=tab2, rhs_sb=hT1_sb, W0=wx0_t, W1=wx1_t, b=bx_t,
                 fo=fhid, hT_next=hT2_sb, rows=rows2, tab_next=tab3,
                 tx_d=tx2_d),
            dict(table=tab3, rhs_sb=hT2_sb, W0=w20_t, W1=w21_t, b=b2_t,
                 fo=fout, hT_next=None, rows=None, tab_next=None,
                 tx_d=tx3_d),
        ]

        for li, L in enumerate(layers):
            fo = L["fo"]
            rdt = I8 if li == 0 else BF16
            # propagate: gather / weight / reduce per tile group
            for (t0, tt, m, c0) in groups:
                r = rp.tile([128, tt, m, fin], rdt, tag="r")
                for t in range(tt):
                    for j in range(m):
                        nc.gpsimd.indirect_dma_start(
                            out=r[:, t, j, :],
                            out_offset=None,
                            in_=L["table"][:, :],
                            in_offset=bass.IndirectOffsetOnAxis(
                                ap=offs_all[:, c0 + t * m + j:
                                            c0 + t * m + j + 1],
                                axis=0,
                            ),
                        )
                rw = rwp.tile([128, tt, m, fin], BF16, tag="rw")
                nc.vector.tensor_tensor(
                    out=rw[:],
                    in0=r[:],
                    in1=w_u16[:, c0:c0 + tt * m].bitcast(BF16).rearrange(
                        "p (t m) -> p t m", m=m
                    ).unsqueeze(3).broadcast_to([128, tt, m, fin]),
                    op=mybir.AluOpType.mult,
                )
                txo = txop.tile([128, tt, fin], BF16, tag="txo")
                with nc.allow_low_precision(
                    reason="segment sums of <=40 bf16 terms; tolerance 2e-2"
                ):
                    nc.vector.tensor_reduce(
                        out=txo[:],
                        in_=rw[:].rearrange("p t m f -> p t f m"),
                        axis=mybir.AxisListType.X,
                        op=mybir.AluOpType.add,
                    )
                nc.sync.dma_start(
                    out=L["tx_d"][t0 * 128:(t0 + tt) * 128, :].rearrange(
                        "(t p) f -> p t f", p=128
                    ),
                    in_=txo[:],
                )
            # whole-layer transpose of segment sums for the dense term
            nc.sync.dma_start_transpose(txT_sb[:], L["tx_d"][:, :])
            # dense + bias/relu, TG node tiles per op
            for q0 in range(0, n_tiles2, TG):
                qt = min(TG, n_tiles2 - q0)
                c0 = q0 * 128
                gw = qt * 128
                pb = psB.tile([fo, TG * 128], F32, tag="pb")
                nc.tensor.matmul(
                    pb[:, :gw], lhsT=L["W0"],
                    rhs=L["rhs_sb"][:, c0:c0 + gw],
                    start=True, stop=False,
                )
                nc.tensor.matmul(
                    pb[:, :gw], lhsT=L["W1"],
                    rhs=txT_sb[:, c0:c0 + gw],
                    start=False, stop=True,
                )
                if L["hT_next"] is not None:
                    osl = L["hT_next"][:, c0:c0 + gw]
                    nc.scalar.activation(
                        osl, pb[:, :gw],
                        mybir.ActivationFunctionType.Relu,
                        bias=L["b"][:],
                    )
                    nc.sync.dma_start(
                        out=L["rows"][c0:c0 + gw, :].rearrange(
                            "n f -> f n"
                        ),
                        in_=osl,
                    )
                else:
                    ot = otp.tile([fout, TG * 128], BF16, tag="ot")
                    nc.scalar.activation(
                        ot[:, :gw], pb[:, :gw],
                        mybir.ActivationFunctionType.Identity,
                        bias=L["b"][:],
                    )
                    nc.sync.dma_start(
                        out=out_d[:, c0:c0 + gw], in_=ot[:, :gw]
                    )
            if L["tab_next"] is not None:
                nc.gpsimd.collective_compute(
                    "AllGather",
                    mybir.AluOpType.bypass,
                    replica_groups=rg,
                    ins=[L["rows"][:, :]],
                    outs=[L["tab_next"][:, :]],
                )

    nc.compile()
    return nc


# ------------------------------------------------------------------ runner
def make_in_maps(inputs, n_nodes, npc, hp, fin, fhid, fout, xscale):
    n_tiles2 = hp["n_tiles2"]
    npcp2 = hp["npcp2"]
    x = np.asarray(inputs["x"], dtype=np.float32)

    consts = np.zeros((128, CW), dtype=NP_BF16)
    for name, c0 in (("W1_0", W10_C), ("W1_1", W11_C), ("Wx_0", WX0_C),
                     ("Wx_1", WX1_C), ("W2_0", W20_C), ("W2_1", W21_C)):
        w = np.asarray(inputs[name], np.float32)
        if name == "W1_1":
            # gather path feeds integer-valued x/s; fold the scale here
            w = w * xscale
        w = w.astype(NP_BF16)
        consts[0:w.shape[0], c0:c0 + w.shape[1]] = w
    consts[0:fhid, B1_C] = np.asarray(inputs["b1"], np.float32).astype(
        NP_BF16
    )
    consts[0:fhid, BX_C] = np.asarray(inputs["bx"], np.float32).astype(
        NP_BF16
    )
    consts[0:fout, B2_C] = np.asarray(inputs["b2"], np.float32).astype(
        NP_BF16
    )

    in_maps = []
    for c in range(M_CORES):
        xp = np.zeros((npcp2, 4 * XW), dtype=np.float32)
        xp[hp["new_of_old"][c], :fin] = x[c * npc:(c + 1) * npc]
        xq = np.clip(np.round(xp / xscale), -127, 127).astype(np.int8)
        xtiles = np.ascontiguousarray(
            xq.reshape(n_tiles2, 128, 4 * XW).transpose(1, 0, 2).reshape(
                128, n_tiles2 * 4 * XW
            )
        )
        payload = np.concatenate(
            [xtiles.view(np.int32), consts.view(np.int32)], axis=1
        )
        blob = np.concatenate([hp["per_core"][c]["packed"], payload],
                              axis=1)
        in_maps.append(dict(blob=np.ascontiguousarray(blob)))
    return in_maps


def run(inputs, n_nodes, fin, fhid, fout, trace=False, trace_kwargs=None,
        timeit=0):
    npc = n_nodes // M_CORES

    adj = np.asarray(inputs["adj"], dtype=np.int32)
    hp = host_prep(adj, n_nodes, npc)
    xscale = float(np.abs(np.asarray(inputs["x"])).max() / 127.0)
    nc = build_program(hp, fin, fhid, fout, xscale)
    in_maps = make_in_maps(inputs, n_nodes, npc, hp, fin, fhid, fout,
                           xscale)
    res = run_bass_kernel_spmd(
        nc,
        in_maps,
        core_ids=list(range(M_CORES)),
        trace=trace,
        **(trace_kwargs or {}),
    )
    times = []
    for _ in range(timeit):
        t0 = time.perf_counter()
        run_bass_kernel_spmd(nc, in_maps, core_ids=list(range(M_CORES)))
        times.append(time.perf_counter() - t0)
    if times:
        print("repeat wall times (s):", [f"{t:.3f}" for t in times])
        global LAST_TIMES
        LAST_TIMES = times
    out = np.concatenate(
        [
            np.asarray(res.results[c]["out"])[:, hp["new_of_old"][c]]
            .T.astype(np.float32)
            for c in range(M_CORES)
        ],
        axis=0,
    )
    return out, res


def kernel(**inputs):
    out, _ = run(inputs, n_nodes=100000, fin=64, fhid=64, fout=16)
    return out
